# revision 26
# baseline (speedup 1.0000x reference)
"""Trainium2 Bass kernel for nn_MultiHeadAttention_84052509983469 (v2).

Full-input contract: kernel(**inputs) takes the complete tensors and
returns the complete [B, S, D] output. Work is sharded across 8 cores as
(batch b in {0,1}) x (head-group g in {0..3}): 4 heads / 256 features per
core, data-parallel over batch. Per core:

  Q^T,K^T = W{q,k}_g @ x_b^T   (bf16; head dims pre-permuted to
                                [even...,odd...] per head on host)
  RoPE    : qt = qt*cos2 + swap32(qt)*sin2s, with swap32 done on the PE by
            a one-hot permutation matmul (row i <-> i^32) and the sign
            pattern folded into the sin table on host.
  V_aug   = gated V plus a gated ones-column (softmax denominator)
  S^T     = K_h^T.T @ Q_h^T    (scores transposed: keys on partitions)
  P^T     = exp(S^T/8) * diag-causal-mask  (bf16)
  O^T     = V_aug^T @ P^T      (row 64 = denominator)
  denom   : Pool-copy of the denominator rows -> DVE reciprocal ->
            PE broadcast (ones2 @ rc) to all 128 partitions -> Pool evict
  attn^T  = O^T[0:64] * bcast(1/denom)  (bf16)
  partial = attn^T.T @ Wo_g^T  (row-sharded Wo; PSUM evicted by Pool to
            bf16 and DMA'd out; interleaved into the attention loop)

Host gathers: out[b] = sum_g partial[b,g] + bo + bv @ Wo^T.
Everything on-device is bf16 except PSUM accumulation and the softmax
denominator (fp32); measured end-to-end rms error vs the fp32 reference
is ~6e-3 (gate: 2e-2).
"""

import sys

if "/opt/trn_rl_repo" not in sys.path:
    sys.path.insert(0, "/opt/trn_rl_repo")

import numpy as np

import concourse.bass as bass
import concourse.mybir as mybir
import concourse.tile as tile
from concourse import bacc

# Problem shapes
B, S, D, H = 2, 2048, 1024, 16
HD = D // H  # 64
G = 4  # head groups (cores per batch)
HL = H // G  # heads per core = 4
GF = HL * HD  # features per core = 256
P = 128
NK = S // P  # 16 key tiles
NQ = 256  # query chunk size
NQC = S // NQ  # 8 query chunks
KT = D // P  # 8 contraction tiles for projections

F32 = mybir.dt.float32
F32R = mybir.dt.float32r
BF16 = mybir.dt.bfloat16


def build_nc(repeat=1):
    nc = bacc.Bacc(None, target_bir_lowering=False, debug=False)

    # ---- DRAM I/O (host supplies pre-tiled bf16 layouts) ----
    xt = nc.dram_tensor("xt", [P, KT, S], BF16, kind="ExternalInput")  # x^T tiles
    wq = nc.dram_tensor("wq", [P, KT, GF], BF16, kind="ExternalInput")
    wk = nc.dram_tensor("wk", [P, KT, GF], BF16, kind="ExternalInput")
    wv = nc.dram_tensor("wv", [P, KT, GF], BF16, kind="ExternalInput")
    wo = nc.dram_tensor("wo", [P, 2, D], BF16, kind="ExternalInput")  # Wo^T rows
    bq = nc.dram_tensor("bq", [P, 2], F32, kind="ExternalInput")
    bk = nc.dram_tensor("bk", [P, 2], F32, kind="ExternalInput")
    ct = nc.dram_tensor("ct", [P, S], BF16, kind="ExternalInput")  # cos, tiled x4
    st = nc.dram_tensor("st", [P, S], BF16, kind="ExternalInput")  # sin, sign-folded
    psw = nc.dram_tensor("psw", [P, P], BF16, kind="ExternalInput")  # i<->i^32 perm
    gate = nc.dram_tensor("gate", [P, NK], F32, kind="ExternalInput")
    cmask = nc.dram_tensor("cmask", [P, NQ // P, NQ], BF16, kind="ExternalInput")
    out = nc.dram_tensor("out", [P, NK, D], BF16, kind="ExternalOutput")

    with tile.TileContext(nc) as tc:
        with (
            tc.tile_pool(name="xtp", bufs=1) as xtp,
            tc.tile_pool(name="wp", bufs=1) as wp,
            tc.tile_pool(name="const", bufs=1) as constp,
            tc.tile_pool(name="qk", bufs=1) as qkp,
            tc.tile_pool(name="ropea", bufs=3) as ropap,
            tc.tile_pool(name="ropeb", bufs=3) as robp,
            tc.tile_pool(name="vaug", bufs=1) as vaugp,
            tc.tile_pool(name="psum", bufs=1, space="PSUM") as pp,
            tc.tile_pool(name="pexp", bufs=6) as pexpp,
            tc.tile_pool(name="dnm", bufs=3) as dnmp,
            tc.tile_pool(name="ob", bufs=6) as obp,
        ):
            # PSUM budget (16KB/partition = 8 banks):
            #   tag "s4" 4KB x2 =  8KB : proj acc (A) / score groups (C)
            #   tag "k2" 2KB x2 =  4KB : rope perm (A) / denom bcast + out-proj
            #   tag "k1" 2KB x2 =  4KB : V acc (B) / attention out pair (C)
            def ps_s4(name, shape, padded):
                return pp.tile(shape, F32, tag="s4", bufs=2,
                               padded_shape=padded, name=name)

            def ps_k2(name, shape, padded):
                return pp.tile(shape, F32, tag="k2", bufs=2,
                               padded_shape=padded, name=name)

            def ps_k1(name, shape, padded):
                return pp.tile(shape, F32, tag="k1", bufs=2,
                               padded_shape=padded, name=name)

            for _rep in range(repeat):
                # ---- load constants / inputs to SBUF ----
                xt_sb = xtp.tile([P, KT, S], BF16, tag="xt")
                wq_sb = wp.tile([P, KT, GF], BF16, tag="wq")
                wk_sb = wp.tile([P, KT, GF], BF16, tag="wk")
                wv_sb = wp.tile([P, KT, GF], BF16, tag="wv")
                bq_sb = constp.tile([P, 2], F32, tag="bq")
                bk_sb = constp.tile([P, 2], F32, tag="bk")
                psw_sb = constp.tile([P, P], BF16, tag="psw")
                # order by first use: Q-proj weights and the first xt columns
                # gate the first matmuls
                # three parallel load queues: SP carries what phase A's
                # projections need first; Pool carries the back halves and
                # phase B/C constants; Act carries the rope tables
                nc.sync.dma_start(out=wq_sb[:], in_=wq[:])
                HS2 = S // 2
                qs = {0: nc.sync, 1: nc.scalar, 2: nc.gpsimd}
                for a in range(KT):
                    qs[a % 3].dma_start(out=xt_sb[:, a, 0:HS2], in_=xt[:, a, 0:HS2])
                nc.sync.dma_start(out=bq_sb[:], in_=bq[:])
                nc.sync.dma_start(out=psw_sb[:], in_=psw[:])
                nc.sync.dma_start(out=wk_sb[:], in_=wk[:])
                nc.sync.dma_start(out=bk_sb[:], in_=bk[:])
                for a in range(KT):
                    qs[a % 2 + 1].dma_start(out=xt_sb[:, a, HS2:S], in_=xt[:, a, HS2:S])
                ct_sb = constp.tile([P, S], BF16, tag="ct")
                st_sb = constp.tile([P, S], BF16, tag="st")
                nc.scalar.dma_start(out=ct_sb[:], in_=ct[:])
                nc.scalar.dma_start(out=st_sb[:], in_=st[:])
                gate_sb = constp.tile([P, NK], F32, tag="gate")
                nc.gpsimd.dma_start(out=wv_sb[:], in_=wv[:])
                nc.gpsimd.dma_start(out=gate_sb[:], in_=gate[:])
                cm_sb = constp.tile([P, NQ // P, NQ], BF16, tag="cmask")
                nc.gpsimd.dma_start(out=cm_sb[:], in_=cmask[:])
                wo_sb = wp.tile([P, 2, D], BF16, tag="wo")
                nc.gpsimd.dma_start(out=wo_sb[:], in_=wo[:])
                onesh = constp.tile([P, 4], BF16, tag="onesh")
                nc.vector.memset(onesh[:], 1.0)
                # single-partition ones row for the denominator broadcast
                ones1 = constp.tile([1, P], BF16, tag="ones1")
                nc.vector.memset(ones1[:], 1.0)

                # ---- Phase A: Q^T / K^T projections (+bias) then RoPE ----
                # qk[i]: [128, S] bf16, i in (q_m0, q_m1, k_m0, k_m1); rows =
                # head-feature (2 heads per m-tile; per head: 32 even dims
                # then 32 odd dims).
                qk = [
                    qkp.tile([P, S], BF16, tag=f"qk{i}", name=f"qk{i}")
                    for i in range(4)
                ]
                def proj(i):
                    w_sb, b_sb = ((wq_sb, bq_sb), (wq_sb, bq_sb),
                                  (wk_sb, bk_sb), (wk_sb, bk_sb))[i]
                    m = i % 2
                    tgt = qk[i]
                    for c4 in range(S // 512):
                        cl = slice(c4 * 512, (c4 + 1) * 512)
                        ps = ps_s4("ps_proj", [P, 512], [P, 1024])
                        for k in range(KT):
                            nc.tensor.matmul(
                                ps[:],
                                w_sb[:, k, m * P : (m + 1) * P],
                                xt_sb[:, k, cl],
                                start=(k == 0),
                                stop=(k == KT - 1),
                            )
                        nc.scalar.activation(
                            out=tgt[:, cl],
                            in_=ps[:],
                            func=mybir.ActivationFunctionType.Identity,
                            bias=b_sb[:, m : m + 1],
                            scale=1.0,
                        )

                def rope(i):
                    # RoPE: swap32 on PE, evict on Pool, combine on DVE
                    tgt = qk[i]
                    for c4 in range(S // 512):
                        cl = slice(c4 * 512, (c4 + 1) * 512)
                        pp_ = ps_k2("ps_perm", [P, 512], [P, 512])
                        nc.tensor.matmul(
                            pp_[:], psw_sb[:], tgt[:, cl], start=True, stop=True
                        )
                        bs = robp.tile([P, 512], BF16, tag="bs", name="bs")
                        nc.scalar.copy(out=bs[:], in_=pp_[:])
                        nc.gpsimd.tensor_mul(bs[:], bs[:], st_sb[:, cl])
                        a_t = ropap.tile([P, 512], BF16, tag="ra", name="a_t")
                        nc.vector.tensor_mul(a_t[:], tgt[:, cl], ct_sb[:, cl])
                        nc.vector.tensor_add(tgt[:, cl], a_t[:], bs[:])

                # stagger: each target's perm lands one projection later
                # (Act evictions ready), and the m0 pair (q_m0, k_m0) finishes
                # first so chunk 0's h0/h1 dependencies resolve early
                proj(0)
                proj(2)
                rope(0)
                proj(1)
                rope(2)
                proj(3)
                rope(1)
                rope(3)

                # ---- Phase B: V projection (emitted just-in-time per
                # chunk inside the attention loop) ----
                vaug = [
                    vaugp.tile([P, HL, HD + 1], BF16, tag=f"vaug{t}", name=f"vaug{t}")
                    for t in range(NK)
                ]

                def v_proj(t):
                    ps = ps_k1("ps_vproj", [P, GF], [P, 512])
                    for k in range(KT):
                        nc.tensor.matmul(
                            ps[:],
                            xt_sb[:, k, t * P : (t + 1) * P],
                            wv_sb[:, k, :],
                            start=(k == 0),
                            stop=(k == KT - 1),
                        )
                    nc.vector.tensor_scalar_mul(
                        vaug[t][:, :, 0:HD],
                        ps[:].rearrange("p (h d) -> p h d", h=HL),
                        gate_sb[:, t : t + 1],
                    )
                    nc.gpsimd.tensor_scalar_mul(
                        vaug[t][:, :, HD : HD + 1],
                        onesh[:].unsqueeze(-1),
                        gate_sb[:, t : t + 1],
                    )

                # ---- Phase C: attention, with Phase D (out proj) interleaved ----
                attnt = [
                    qkp.tile([P, S], BF16, tag="attnt0", name="attnt0"),
                    qkp.tile([P, S], BF16, tag="attnt1", name="attnt1"),
                ]

                def out_proj(t):
                    for oc in range(2):
                        psd = ps_k2("ps_oproj", [P, 512], [P, 512])
                        for m in range(2):
                            nc.tensor.matmul(
                                psd[:],
                                attnt[m][:, t * P : (t + 1) * P],
                                wo_sb[:, m, oc * 512 : (oc + 1) * 512],
                                start=(m == 0),
                                stop=(m == 1),
                            )
                        ob = obp.tile([P, 512], BF16, tag="ob", name="ob")
                        nc.vector.tensor_copy(out=ob[:], in_=psd[:])
                        nc.sync.dma_start(
                            out=out[:, t, oc * 512 : (oc + 1) * 512], in_=ob[:]
                        )

                # Software-pipelined attention: per chunk, flatten the
                # (head, key-tile-group) work into a list, then emit
                # scores(j+1) before PV(j) so the PE always has a score
                # group in flight while the Act engine runs exp(j).
                for c in range(NQC):
                    v_proj(2 * c)
                    v_proj(2 * c + 1)
                    q0, q1 = c * NQ, (c + 1) * NQ
                    nt = 2 * c + 2  # causal: key tiles 0 .. 2c+1
                    items = []
                    for h in range(HL):
                        t0 = 0
                        while t0 < nt:
                            gw = min(4, nt - t0)
                            items.append((h, t0, gw))
                            t0 += gw

                    po2s = {}
                    pes = {}

                    def scores(j):
                        h, t0, gw = items[j]
                        m, r0 = h // 2, 64 * (h % 2)
                        if h % 2 == 0 and t0 == 0:
                            po2s[h // 2] = ps_k1(
                                "ps_att", [HD + 1, 2, NQ], [HD + 1, 2, NQ]
                            )
                        ps = ps_s4("ps_s", [P, gw, NQ], [P, 4, NQ])
                        for u in range(gw):
                            nc.tensor.matmul(
                                ps[:, u, :],
                                qk[2 + m][
                                    r0 : r0 + 64, (t0 + u) * P : (t0 + u + 1) * P
                                ],
                                qk[m][r0 : r0 + 64, q0:q1],
                                start=True,
                                stop=True,
                            )
                        pe = pexpp.tile(
                            [P, gw, NQ], BF16, tag="pexp",
                            padded_shape=[P, 4, NQ],
                        )
                        nc.scalar.activation(
                            out=pe[:], in_=ps[:],
                            func=mybir.ActivationFunctionType.Exp,
                            scale=float(1.0 / np.sqrt(HD)),
                        )
                        if t0 + gw == nt:  # last group holds the diagonal pair
                            nc.gpsimd.tensor_mul(
                                pe[:, gw - 2 : gw, :],
                                pe[:, gw - 2 : gw, :],
                                cm_sb[:],
                            )
                        pes[j] = pe

                    def pv(j):
                        h, t0, gw = items[j]
                        m = h // 2
                        po2 = po2s[m]
                        po = po2[:, h % 2, :]
                        pe = pes.pop(j)
                        for u in range(gw):
                            t = t0 + u
                            nc.tensor.matmul(
                                po[:],
                                vaug[t][:, h, :],
                                pe[:, u, :],
                                start=(t == 0),
                                stop=(t == nt - 1),
                            )
                        if t0 + gw < nt:
                            return
                        if h % 2 == 1:
                            # both heads of this m-tile done: one batched
                            # reciprocal of the two denominator rows straight
                            # from PSUM, a rank-1 broadcast (every output row
                            # is [1/d_h0 | 1/d_h1]), evict, normalize
                            rc = dnmp.tile([1, 2, NQ], BF16, tag="rc", name="rc")
                            with nc.allow_low_precision(
                                reason="1/denom rounded to bf16 feeds a "
                                "rank-1 broadcast matmul; 0.4% on [0,1]"
                            ):
                                nc.vector.reciprocal(
                                    rc[:].rearrange("p h q -> p (h q)"),
                                    po2[HD : HD + 1, :, :].rearrange(
                                        "p h q -> p (h q)"
                                    ),
                                )
                            rb = ps_k2("ps_rb", [P, 2, NQ], [P, 2, NQ])
                            nc.tensor.matmul(
                                rb[:].rearrange("p h q -> p (h q)"),
                                ones1[:],
                                rc[:].rearrange("p h q -> p (h q)"),
                                start=True,
                                stop=True,
                            )
                            rbs = dnmp.tile([P, 2, NQ], BF16, tag="rbs", name="rbs")
                            nc.vector.tensor_copy(out=rbs[:], in_=rb[:])
                            for hh in (0, 1):
                                rr = 64 * hh
                                nc.vector.tensor_mul(
                                    attnt[m][rr : rr + 64, q0:q1],
                                    po2[0:HD, hh, :],
                                    rbs[rr : rr + 64, hh, :],
                                )

                    dsbs = {}
                    scores(0)
                    for j in range(1, len(items)):
                        scores(j)
                        pv(j - 1)
                    pv(len(items) - 1)
                    # interleave the previous chunk's output-projection tiles
                    if c > 0:
                        out_proj(2 * (c - 1))
                        out_proj(2 * (c - 1) + 1)
                out_proj(NK - 2)
                out_proj(NK - 1)
    nc.compile()
    return nc


# ---------------- host-side prep ----------------

_PERM64 = np.concatenate([np.arange(0, HD, 2), np.arange(1, HD, 2)])


def _np_bf16():
    import ml_dtypes

    return np.dtype(ml_dtypes.bfloat16)


def _rope_tables():
    inv = 1.0 / (10000.0 ** (np.arange(0, HD, 2, dtype=np.float32) / HD))
    t = np.arange(S, dtype=np.float32)
    ang = np.outer(t, inv)  # [S, HD/2]
    return np.cos(ang).astype(np.float32), np.sin(ang).astype(np.float32)


def _tile_rows(a, p=P):
    """[R, N] -> [p, R//p, N]: row r of the result = a[... * p + r]"""
    R = a.shape[0]
    return np.ascontiguousarray(
        a.reshape(R // p, p, *a.shape[1:]).transpose(1, 0, *range(2, a.ndim + 1))
    )


def shard_inputs(x, effective_len, Wq, bq, Wk, bk, Wv, bv, Wo, bo):
    bf16 = _np_bf16()
    x = np.asarray(x, np.float32)
    effective_len = np.asarray(effective_len, np.int32)
    Wq, Wk, Wv, Wo = (np.asarray(w, np.float32) for w in (Wq, Wk, Wv, Wo))
    bq, bk = (np.asarray(b, np.float32) for b in (bq, bk))

    cos, sin = _rope_tables()  # [S, 32]
    ct = np.ascontiguousarray(np.tile(cos.T, (4, 1))).astype(bf16)  # [128, S]
    # sign-folded sin: rows 0-31 of each 64-block get -sin (even dests),
    # rows 32-63 get +sin (odd dests)
    sgn = np.tile(np.repeat(np.array([-1.0, 1.0], np.float32), 32), 2)
    st = np.ascontiguousarray(np.tile(sin.T, (4, 1)) * sgn[:, None]).astype(bf16)

    # swap32 permutation: out row i = in row (i ^ 32)
    pswap = np.zeros((P, P), np.float32)
    pswap[np.arange(P), np.arange(P) ^ 32] = 1.0
    pswap = np.ascontiguousarray(pswap).astype(bf16)

    # causal multiplicative masks for the two diagonal key-tiles of a chunk
    kl = np.arange(P)[:, None]
    ql = np.arange(NQ)[None, :]
    cm = np.stack(
        [(ql >= kl + P * j).astype(np.float32) for j in range(NQ // P)]
    )  # [NQ//P, 128, NQ]
    cmask = np.ascontiguousarray(cm.transpose(1, 0, 2)).astype(bf16)

    in_maps = []
    for b in range(B):
        xt = _tile_rows(np.ascontiguousarray(x[b].T)).astype(bf16)  # [128, 8, S]
        g_vec = (np.arange(S) < (S - int(effective_len[b]))).astype(np.float32)
        gate = np.ascontiguousarray(g_vec.reshape(NK, P).T)  # [128, NK]
        for g in range(G):
            rows = np.concatenate(
                [g * GF + h * HD + _PERM64 for h in range(HL)]
            )  # permuted head dims for Q/K
            vrows = np.arange(g * GF, (g + 1) * GF)
            in_maps.append(
                {
                    "xt": xt,
                    "wq": _tile_rows(np.ascontiguousarray(Wq[rows].T)).astype(bf16),
                    "wk": _tile_rows(np.ascontiguousarray(Wk[rows].T)).astype(bf16),
                    "wv": _tile_rows(np.ascontiguousarray(Wv[vrows].T)).astype(bf16),
                    "wo": _tile_rows(np.ascontiguousarray(Wo[:, vrows].T)).astype(
                        bf16
                    ),
                    "bq": np.ascontiguousarray(bq[rows].reshape(2, P).T),
                    "bk": np.ascontiguousarray(bk[rows].reshape(2, P).T),
                    "ct": ct,
                    "st": st,
                    "psw": pswap,
                    "gate": gate,
                    "cmask": cmask,
                }
            )
    return in_maps


def gather_outputs(results, bo, bv_wo=0.0):
    """bv_wo: precomputed bv @ Wo.T correction row (bv commutes
    through the output projection as a constant)."""
    bo = np.asarray(bo, np.float32)
    out = np.zeros((B, S, D), np.float32)
    for b in range(B):
        acc = np.zeros((S, D), np.float32)
        for g in range(G):
            o3 = results[b * G + g]["out"].astype(np.float32)  # [128, NK, D]
            acc += o3.transpose(1, 0, 2).reshape(S, D)
        out[b] = acc + bo + bv_wo
    return out


_NC_CACHE = None


def _get_nc():
    global _NC_CACHE
    if _NC_CACHE is None:
        _NC_CACHE = build_nc()
    return _NC_CACHE


def kernel(**inputs):
    from concourse.bass_utils import run_bass_kernel_spmd

    nc = _get_nc()
    in_maps = shard_inputs(**inputs)
    res = run_bass_kernel_spmd(nc, in_maps, core_ids=list(range(8)))
    bv_wo = np.asarray(inputs["bv"], np.float32) @ np.asarray(
        inputs["Wo"], np.float32
    ).T
    return gather_outputs(res.results, inputs["bo"], bv_wo)
